# revision 28
# baseline (speedup 1.0000x reference)
"""Trainium2 Bass kernel for nn_Attn_30820685316537 (segment_reduce attention).

Reference computation (per batch b):
    score = output @ context^T                     [Q, S]
    avg   = per-segment mean of score over S, broadcast back
    align = softmax(avg, axis=S)                   [Q, S]
    ac    = align @ context                        [Q, D]
    out   = tanh(concat(ac, output) @ W^T + bias)  [Q, D]
    returns (out, align)

Everything factors through rank-64 segment space (avg is constant within each
contiguous segment).  With Cavg[n, d] = (1/cnt_n) * sum_{s in seg n} C[s, d]:
    segavg[q, n] = O[q, :] . Cavg[n, :]
    u[q, n]      = exp(segavg + ln cnt_n - max)        (cnt-weighted softmax)
    urn[q, n]    = u / sum_n u                         (per-segment align mass)
    align[q, s]  = urn[q, seg(s)] / cnt_{seg(s)}       (host-side gather)
    ac[q, :]     = urn @ Cavg
    out          = tanh(urn @ (Cavg @ W1) + O @ W2 + bias)
where W^T = [W1; W2].  P = Cavg @ W1 is a [64, D] matrix, so the output GEMM
is O @ W2 (K=1024) plus a K=65 rank-64 correction (bias folded in as P's 65th
row) instead of the K=2048 concat GEMM.

Schedule: all inputs are host-packed into their final SBUF layouts and
streamed need-ordered, each tensor split across the two HW-DGE rings (sync +
scalar) so single-tensor arrival time is halved; identh/bias + w2-hi ride the
software-DGE gpsimd ring.  A memset-backed junk-matmul block warms the PE HAM
clock-gate before any input lands, and small keep-alive matmul groups are
placed at every known input-wait point so the clock never re-throttles during
the input-paced ramp.  Per batch: csum (one-hot matmul, invc pre-folded on
host) -> 8 PE transposes -> segavg -> per-q-tile softmax (DVE/ACT) -> urT
transposes; P = CavgT^T @ W1 in fp8 (x4096 scaling).  qloop per 128-row
q-tile: O@W2 lo then hi columns; urn applied (K=65) into the same PSUM
accumulators; tanh + DMA out per half.  Two batches per core are
software-pipelined; GEMMs fp16 (fp32 PSUM), P-path fp8.  align is
reconstructed on the host by a pure gather of the device-computed urn masses.

Sharding: data-parallel over batch B=16 across 8 NeuronCores; W replicated.
"""
import numpy as np
from contextlib import ExitStack

B, Q, S, D = 16, 512, 1024, 1024
NSEG = 64
NCORES = 8
BPC = B // NCORES          # batches per core
QT = Q // 128              # 4 q-tiles
ST = S // 128              # 8 s-chunks
DT = D // 128              # 8 d-chunks

_CACHE = {}


def _build_nc():
    import concourse.bacc as bacc
    import concourse.tile as tile
    import concourse.mybir as mybir

    f32 = mybir.dt.float32
    f16 = mybir.dt.float16
    f8 = mybir.dt.float8e4

    nc = bacc.Bacc("TRN2", target_bir_lowering=False, debug=False,
                   enable_asserts=False, num_devices=NCORES)

    # All inputs host-packed to final SBUF layout [128, X].
    ctx_in = nc.dram_tensor("ctx_in", [BPC, 128, ST * D], f16, kind="ExternalInput")
    ott_in = nc.dram_tensor("ott_in", [BPC, 128, DT * Q], f16, kind="ExternalInput")
    w1_in = nc.dram_tensor("w1_in", [128, DT * D], f8, kind="ExternalInput")
    w2lo_in = nc.dram_tensor("w2lo_in", [128, DT * 512], f16, kind="ExternalInput")
    w2hi_in = nc.dram_tensor("w2hi_in", [128, DT * 512], f16, kind="ExternalInput")
    ohi_in = nc.dram_tensor("ohi_in", [BPC, 128, ST * NSEG], f16, kind="ExternalInput")
    lnc_in = nc.dram_tensor("lnc_in", [BPC, 128, NSEG], f32, kind="ExternalInput")
    bias_in = nc.dram_tensor("bias_in", [1, D], f16, kind="ExternalInput")
    identh_in = nc.dram_tensor("identh_in", [128, 128], f16, kind="ExternalInput")

    out_o = nc.dram_tensor("out_o", [BPC, Q, D], f16, kind="ExternalOutput")
    urn_o = nc.dram_tensor("urn_o", [BPC, Q, NSEG], f16, kind="ExternalOutput")

    Exp = mybir.ActivationFunctionType.Exp
    Tanh = mybir.ActivationFunctionType.Tanh
    Xax = mybir.AxisListType.X

    with tile.TileContext(nc) as tc, ExitStack() as ectx:
        consts = ectx.enter_context(tc.tile_pool(name="consts", bufs=1))
        inp = ectx.enter_context(tc.tile_pool(name="inp", bufs=2))
        front = ectx.enter_context(tc.tile_pool(name="front", bufs=2))
        sm = ectx.enter_context(tc.tile_pool(name="sm", bufs=3))
        stage = ectx.enter_context(tc.tile_pool(name="stage", bufs=2))

        # PSUM: exactly 8 banks (2 + 1 + 2 + 2 + 1 junk).
        ps64 = ectx.enter_context(tc.tile_pool(name="ps64", bufs=2, space="PSUM"))
        ps_t = ectx.enter_context(tc.tile_pool(name="ps_t", bufs=1, space="PSUM"))
        ps_lo = ectx.enter_context(tc.tile_pool(name="ps_lo", bufs=2, space="PSUM"))
        ps_hi = ectx.enter_context(tc.tile_pool(name="ps_hi", bufs=2, space="PSUM"))
        ps_j = ectx.enter_context(tc.tile_pool(name="ps_j", bufs=1, space="PSUM"))

        # ---- HAM warmup: junk weights via memset (no DMA dependency) ----
        junkw = consts.tile([128, 128], f16, tag="junkw")
        nc.vector.memset(junkw[:], 0.0)
        junkp = ps_j.tile([128, 128], f32, tag="junk", name="junkp")

        def emit_warmup(n, group=True):
            # n back-to-back junk matmuls (N=128) to hold the HAM clock-gate
            # at full rate; `group=True` makes them one accumulation group
            # (single semaphore).
            for r in range(n):
                nc.tensor.matmul(junkp[:], junkw[:], junkw[:],
                                 start=(r == 0 if group else True),
                                 stop=(r == n - 1 if group else True))

        # ---- const/weight tiles ----
        identh = consts.tile([128, 128], f16, tag="identh")
        bias_sb = consts.tile([1, D], f16, tag="bias")
        w1_all = consts.tile([128, DT * D], f8, tag="w1")        # [p, (d f)] fp8
        w2_all = consts.tile([128, 2 * DT * 512], f16, tag="w2")  # lo cols then hi cols
        w2lo = w2_all[:, 0:DT * 512]
        w2hi = w2_all[:, DT * 512:]

        state = [dict() for _ in range(BPC)]

        def alloc_inputs(b):
            st = state[b]
            st["ohi"] = inp.tile([128, ST * NSEG], f16, tag="ohi", name=f"ohi{b}")
            st["lnc"] = inp.tile([128, NSEG], f32, tag="lnc", name=f"lnc{b}")
            st["ctx"] = inp.tile([128, ST * D], f16, tag="ctx", name=f"ctx{b}")
            st["ott"] = inp.tile([128, DT * Q], f16, tag="ott", name=f"ott{b}")

        alloc_inputs(0)
        alloc_inputs(1)

        # ---- DMA schedule: need-ordered, split across both HW rings ----
        # sync ring: first half of every bulk tensor; scalar ring: ohi/lnc
        # first, then second halves.  gpsimd (SW DGE): identh/bias + w2-hi.
        sy, sc, gp = nc.sync, nc.scalar, nc.gpsimd

        gp.dma_start(identh[:], identh_in.ap())
        gp.dma_start(bias_sb[:], bias_in.ap())

        st0, st1 = state[0], state[1]
        sc.dma_start(st0["ohi"][:], ohi_in.ap()[0])
        sc.dma_start(st0["lnc"][:], lnc_in.ap()[0])

        # ctx0: 4 calls of 2 s-chunks each; sync gets chunks 0-3, scalar 4-7.
        for i, eng in ((0, sy), (2, sy), (4, sc), (6, sc)):
            eng.dma_start(st0["ctx"][:, D * i:D * (i + 2)],
                          ctx_in.ap()[0][:, D * i:D * (i + 2)])
        # ohi1 + ctx1 ride directly behind ctx0: batch-1 csum is the only
        # sizeable PE work whose input can arrive early enough to fill the
        # ramp gap while ott0/w1/w2 are still streaming.
        sc.dma_start(st1["ohi"][:], ohi_in.ap()[1])
        sc.dma_start(st1["lnc"][:], lnc_in.ap()[1])
        for i, eng in ((0, sy), (2, sy), (4, sc), (6, sc)):
            eng.dma_start(st1["ctx"][:, D * i:D * (i + 2)],
                          ctx_in.ap()[1][:, D * i:D * (i + 2)])
        # ott0 halves
        sy.dma_start(st0["ott"][:, 0:4 * Q], ott_in.ap()[0][:, 0:4 * Q])
        sc.dma_start(st0["ott"][:, 4 * Q:], ott_in.ap()[0][:, 4 * Q:])
        # w1 halves
        sy.dma_start(w1_all[:, 0:4 * D], w1_in.ap()[:, 0:4 * D])
        sc.dma_start(w1_all[:, 4 * D:], w1_in.ap()[:, 4 * D:])
        # w2 lo halves (qlo pass), hi on gpsimd (needed ~6us later)
        sy.dma_start(w2lo[:, 0:4 * 512], w2lo_in.ap()[:, 0:4 * 512])
        sc.dma_start(w2lo[:, 4 * 512:], w2lo_in.ap()[:, 4 * 512:])
        gp.dma_start(w2hi[:, 0:4 * 512], w2hi_in.ap()[:, 0:4 * 512])
        gp.dma_start(w2hi[:, 4 * 512:], w2hi_in.ap()[:, 4 * 512:])
        # batch-1 ott last
        sy.dma_start(st1["ott"][:, 0:4 * Q], ott_in.ap()[1][:, 0:4 * Q])
        sc.dma_start(st1["ott"][:, 4 * Q:], ott_in.ap()[1][:, 4 * Q:])

        def emit_csum_mm(b, i):
            # col-group pair: chunk i's d-lo -> psum rows 0:64 (col group 0),
            # d-hi -> rows 64:128 (col group 1); one matmul span per chunk.
            st = state[b]
            ohi, ctx_all = st["ohi"], st["ctx"]
            if i == 0:
                st["csp"] = ps64.tile([128, 512], f32, tag="a64", name=f"csp{b}")
            csp = st["csp"]
            oh_i = ohi[:, NSEG * i:NSEG * (i + 1)]
            nc.tensor.matmul(csp[0:64, :], oh_i, ctx_all[:, D * i:D * i + 512],
                             start=(i == 0), stop=(i == ST - 1),
                             tile_position=(0, 0))
            nc.tensor.matmul(csp[64:128, :], oh_i, ctx_all[:, D * i + 512:D * (i + 1)],
                             start=(i == 0), stop=(i == ST - 1),
                             tile_position=(0, 64))

        def emit_csum_tail(b):
            # stacked csum [128, 512] (rows 0:64 = Cavg d-lo, 64:128 = d-hi)
            # -> SBUF; all 8 CavgT chunks come from plain matmuls with
            # identity slices (rhs identh[:, 0:64] reads partitions 0:64,
            # identh[:, 64:128] reads 64:128) — no transpose_mode needed.
            st = state[b]
            css = front.tile([128, 512], f16, tag="css", name=f"css{b}")
            nc.vector.tensor_copy(css[:], st["csp"][:])
            pt = ps_t.tile([128, 512], f32, tag="tp", name=f"pt{b}")
            for d in range(4):
                blk = css[:, 128 * d:128 * (d + 1)]
                nc.tensor.matmul(pt[:, 64 * d:64 * (d + 1)],
                                 blk, identh[:, 0:64], start=True, stop=True)
                nc.tensor.matmul(pt[:, 64 * (d + 4):64 * (d + 5)],
                                 blk, identh[:, 64:128], start=True, stop=True)
            csumt = front.tile([128, DT * NSEG], f16, tag="csumt")
            nc.vector.tensor_copy(csumt[:], pt[:])
            st["csumt"] = csumt
            csumt8 = front.tile([128, DT * NSEG], f8, tag="csumt8")
            nc.vector.tensor_scalar_mul(csumt8[:], pt[:], 16.0)
            st["csumt8"] = csumt8

            urt = front.tile([65, Q], f16, tag="urt")
            nc.vector.memset(urt[64:65, :], 1.0)
            st["urt"] = urt

        def emit_segavg(b):
            # segavgT stacked [128, 256]: rows 0:64 = q 0:256, rows 64:128 =
            # q 256:512 (col-group pair per d-chunk, N=256 each); then the
            # four [q, n] logit blocks via identity-slice matmuls.
            st = state[b]
            csumt, ott, lnc = st["csumt"], st["ott"], st["lnc"]
            sgp = ps64.tile([128, 256], f32, tag="a64", name=f"sgp{b}")
            for d in range(DT):
                ct_d = csumt[:, NSEG * d:NSEG * (d + 1)]
                nc.tensor.matmul(sgp[0:64, :], ct_d,
                                 ott[:, Q * d:Q * d + 256],
                                 start=(d == 0), stop=(d == DT - 1),
                                 tile_position=(0, 0))
                nc.tensor.matmul(sgp[64:128, :], ct_d,
                                 ott[:, Q * d + 256:Q * d + 512],
                                 start=(d == 0), stop=(d == DT - 1),
                                 tile_position=(0, 64))
            sgs = front.tile([128, 256], f16, tag="sgt", name=f"sgs{b}")
            nc.vector.tensor_copy(sgs[:], sgp[:])
            sgtT = ps_t.tile([128, 512], f32, tag="tp", name=f"sgtT{b}")
            for jj in range(2):
                blk = sgs[:, 128 * jj:128 * (jj + 1)]
                nc.tensor.matmul(sgtT[:, 64 * jj:64 * (jj + 1)],
                                 blk, identh[:, 0:64], start=True, stop=True)
                nc.tensor.matmul(sgtT[:, 64 * (jj + 2):64 * (jj + 3)],
                                 blk, identh[:, 64:128], start=True, stop=True)
            for j in range(QT):
                sg2 = sm.tile([128, NSEG], f32, tag="sg2")
                nc.vector.tensor_add(sg2[:], sgtT[:, 64 * j:64 * (j + 1)], lnc[:])
                mx = sm.tile([128, 1], f32, tag="mx")
                nc.vector.reduce_max(mx[:], sg2[:], axis=Xax)
                negmx = sm.tile([128, 1], f32, tag="negmx")
                nc.vector.tensor_scalar_mul(negmx[:], mx[:], -1.0)
                u = sm.tile([128, NSEG], f16, tag="u")
                dsum = sm.tile([128, 1], f32, tag="dsum")
                nc.scalar.activation(u[:], sg2[:], Exp, bias=negmx[:],
                                     accum_out=dsum[:])
                rd = sm.tile([128, 1], f32, tag="rd")
                nc.vector.reciprocal(rd[:], dsum[:])
                urn = sm.tile([128, NSEG], f16, tag="urn", bufs=5)
                nc.vector.tensor_scalar_mul(urn[:], u[:], rd[:])
                nc.scalar.dma_start(urn_o.ap()[b, 128 * j:128 * (j + 1), :], urn[:])
                st[f"urn{j}"] = urn

        def emit_urt(b):
            # urn.T via plain matmul against the full identity (cheaper than
            # transpose_mode): out[f, n] = urn[n, f].
            st = state[b]
            urt = st["urt"]
            pu = ps_t.tile([64, 512], f32, tag="tp", name=f"pu{b}")
            for j in range(QT):
                nc.tensor.matmul(pu[0:64, 128 * j:128 * (j + 1)],
                                 st[f"urn{j}"][:], identh[:],
                                 start=True, stop=True)
            nc.vector.tensor_copy(urt[0:64, :], pu[0:64, 0:512])

        def emit_p(b):
            # stacked P psum: rows 0:64 = P[:, 0:512], rows 64:128 =
            # P[:, 512:1024]; the hi half returns to partition base 0 via an
            # identity-shift matmul (lhsT = identh[:, 64:128]).
            st = state[b]
            c8 = st["csumt8"]
            pp = ps64.tile([128, 512], f32, tag="a64", name=f"pp{b}")
            for d in range(DT):
                ct_d = c8[:, NSEG * d:NSEG * (d + 1)]
                nc.tensor.matmul(pp[0:64, :], ct_d, w1_all[:, D * d:D * d + 512],
                                 start=(d == 0), stop=(d == DT - 1),
                                 tile_position=(0, 0))
                nc.tensor.matmul(pp[64:128, :], ct_d,
                                 w1_all[:, D * d + 512:D * (d + 1)],
                                 start=(d == 0), stop=(d == DT - 1),
                                 tile_position=(0, 64))
            ppsb = front.tile([128, 512], f16, tag="ppsb", name=f"ppsb{b}")
            nc.vector.tensor_scalar_mul(ppsb[:], pp[:], 1.0 / 4096.0)
            psh = ps_t.tile([64, 512], f32, tag="tp", name=f"psh{b}")
            nc.tensor.matmul(psh[:], identh[:, 64:128], ppsb[:],
                             start=True, stop=True)
            paug = front.tile([65, D], f16, tag="paug")
            nc.vector.tensor_copy(paug[0:64, 0:512], ppsb[0:64, :])
            nc.vector.tensor_copy(paug[0:64, 512:1024], psh[:])
            nc.vector.tensor_copy(paug[64:65, :], bias_sb[:])
            st["paug"] = paug

        def emit_qlo(b, j):
            st = state[b]
            ott = st["ott"]
            o_lo = ps_lo.tile([128, 512], f32, tag="po_lo")
            for d in range(DT):
                otd = ott[:, Q * d + 128 * j:Q * d + 128 * (j + 1)]
                nc.tensor.matmul(o_lo[:], otd, w2lo[:, 512 * d:512 * (d + 1)],
                                 start=(d == 0), stop=False)
            st[f"q{j}"] = o_lo

        def emit_qhi(b, j):
            st = state[b]
            ott = st["ott"]
            o_hi = ps_hi.tile([128, 512], f32, tag="po_hi")
            for d in range(DT):
                otd = ott[:, Q * d + 128 * j:Q * d + 128 * (j + 1)]
                nc.tensor.matmul(o_hi[:], otd, w2hi[:, 512 * d:512 * (d + 1)],
                                 start=(d == 0), stop=False)
            st[f"qh{j}"] = o_hi

        def emit_aplo(b, j):
            st = state[b]
            o_lo = st[f"q{j}"]
            urt, paug = st["urt"], st["paug"]
            ua = urt[:, 128 * j:128 * (j + 1)]
            nc.tensor.matmul(o_lo[:], ua, paug[:, 0:512], start=False, stop=True)
            ost = stage.tile([128, 512], f16, tag="ostl")
            nc.scalar.activation(ost[:], o_lo[:], Tanh)
            nc.scalar.dma_start(out_o.ap()[b, 128 * j:128 * (j + 1), 0:512], ost[:])

        def emit_aphi(b, j):
            st = state[b]
            o_hi = st[f"qh{j}"]
            urt, paug = st["urt"], st["paug"]
            ua = urt[:, 128 * j:128 * (j + 1)]
            nc.tensor.matmul(o_hi[:], ua, paug[:, 512:1024], start=False, stop=True)
            ost = stage.tile([128, 512], f16, tag="osth")
            nc.scalar.activation(ost[:], o_hi[:], Tanh)
            nc.scalar.dma_start(out_o.ap()[b, 128 * j:128 * (j + 1), 512:1024], ost[:])

        # ---- emission ----
        # Initial warmup: runs from ~3.5us (preamble end) through the DMA
        # startup latency window; HAM un-throttles ~3.4us into the block.
        emit_warmup(40)
        # csum0 chunk MMs; keep-alive junk between chunk-pair waits.
        for i in range(ST):
            emit_csum_mm(0, i)
            if i % 2 == 1 and i < ST - 1:
                emit_warmup(4, group=False)
        emit_csum_tail(0)
        # batch-1 csum streams into the ott0/w1 wait window (ctx1 chunks
        # arrive right behind ctx0)
        for i in range(ST):
            emit_csum_mm(1, i)
            if i % 2 == 1 and i < ST - 1:
                emit_warmup(2, group=False)
        emit_csum_tail(1)
        emit_warmup(6, group=False)   # ott0 wait
        emit_segavg(0)
        emit_warmup(6, group=False)   # w1 wait
        emit_p(0)
        emit_urt(0)
        emit_warmup(6, group=False)   # w2lo wait
        emit_qlo(0, 0)
        emit_aplo(0, 0)
        emit_qlo(0, 1)
        emit_aplo(0, 1)
        emit_qlo(0, 2)
        emit_aplo(0, 2)
        emit_qlo(0, 3)
        emit_aplo(0, 3)
        emit_qhi(0, 0)
        emit_aphi(0, 0)
        emit_qhi(0, 1)
        emit_aphi(0, 1)
        emit_qhi(0, 2)
        emit_aphi(0, 2)
        emit_qhi(0, 3)
        emit_aphi(0, 3)
        emit_segavg(1)
        emit_p(1)
        emit_qlo(1, 0)
        emit_qlo(1, 1)
        emit_urt(1)
        emit_aplo(1, 0)
        emit_qlo(1, 2)
        emit_aplo(1, 1)
        emit_qlo(1, 3)
        emit_aplo(1, 2)
        emit_aplo(1, 3)
        emit_qhi(1, 0)
        emit_aphi(1, 0)
        emit_qhi(1, 1)
        emit_aphi(1, 1)
        emit_qhi(1, 2)
        emit_aphi(1, 2)
        emit_qhi(1, 3)
        emit_aphi(1, 3)

    nc.compile()
    return nc


def _host_prep(output, context, W_weight, W_bias, segment_ids):
    """Shard over batch; fp16 conversion + index/layout prep (no reductions)."""
    import concourse.mybir as mybir
    np_f8 = mybir.dt.np(mybir.dt.float8e4)
    wt = W_weight.T.astype(np.float16)                       # [2D, D]
    w1 = (wt[:D].astype(np.float32) * 256.0).astype(np_f8)   # [D, D] fp8
    w2 = wt[D:]                                              # [D, D] f16

    def packK(a, ncol):
        return np.ascontiguousarray(
            a.reshape(DT, 128, ncol).transpose(1, 0, 2).reshape(128, DT * ncol))
    w1p = packK(w1, D)
    w2lop = packK(w2[:, 0:512], 512)
    w2hip = packK(w2[:, 512:1024], 512)
    biasr = np.ascontiguousarray(W_bias.astype(np.float16)[None, :])
    identh = np.eye(128, dtype=np.float16)

    in_maps, aligns = [], []
    for c in range(NCORES):
        lo = c * BPC
        ohis, lncs, invcs = [], [], []
        for b in range(BPC):
            ids = segment_ids[lo + b].astype(np.int64)       # [S]
            oh = (ids[:, None] == np.arange(NSEG)[None, :]).astype(np.float32)
            cnt = oh.sum(axis=0)                             # [NSEG]
            invc = 1.0 / np.maximum(cnt, 1.0)
            ohi = (oh * invc[None, :]).astype(np.float16)    # [S, NSEG]
            ohis.append(np.ascontiguousarray(
                ohi.reshape(ST, 128, NSEG).transpose(1, 0, 2).reshape(128, ST * NSEG)))
            lnrow = np.where(cnt > 0, np.log(np.maximum(cnt, 1.0)), -1e30)
            lncs.append(np.ascontiguousarray(np.broadcast_to(
                lnrow.astype(np.float32)[None, :], (128, NSEG))))
            invcs.append(invc)
        ctxp = np.stack([np.ascontiguousarray(
            context[lo + b].astype(np.float16).reshape(ST, 128, D)
            .transpose(1, 0, 2).reshape(128, ST * D)) for b in range(BPC)])
        ottp = np.stack([np.ascontiguousarray(
            output[lo + b].astype(np.float16).T.reshape(DT, 128, Q)
            .transpose(1, 0, 2).reshape(128, DT * Q)) for b in range(BPC)])
        in_maps.append({
            "ctx_in": ctxp, "ott_in": ottp,
            "w1_in": w1p, "w2lo_in": w2lop, "w2hi_in": w2hip,
            "bias_in": biasr, "identh_in": identh,
            "ohi_in": np.stack(ohis), "lnc_in": np.stack(lncs),
        })
        aligns.append(invcs)
    return in_maps, aligns


def _run(inputs, trace=False, tmpdir=None):
    from concourse.bass_utils import run_bass_kernel_spmd
    if "nc" not in _CACHE:
        _CACHE["nc"] = _build_nc()
    nc = _CACHE["nc"]
    in_maps, invcs = _host_prep(**inputs)
    kw = {}
    if trace:
        kw = {"trace": True, "tmpdir": tmpdir}
    res = run_bass_kernel_spmd(nc, in_maps, core_ids=list(range(NCORES)), **kw)
    out = np.concatenate(
        [res.results[c]["out_o"].astype(np.float32) for c in range(NCORES)], axis=0)
    # align[q, s] = urn[q, seg(s)] * invc[seg(s)]  — host-side gather/unshard
    seg = inputs["segment_ids"]
    align = np.empty((B, Q, S), dtype=np.float32)
    for c in range(NCORES):
        for b in range(BPC):
            gb = c * BPC + b
            urn = res.results[c]["urn_o"][b].astype(np.float32)   # [Q, NSEG]
            scaled = urn * invcs[c][b][None, :].astype(np.float32)
            align[gb] = scaled[:, seg[gb].astype(np.int64)]
    return (out, align), res


def kernel(output, context, W_weight, W_bias, segment_ids):
    # Force host numpy up front: if the caller hands us jax arrays, numpy
    # ops would otherwise dispatch to the accelerator backend.
    (out, align), _ = _run(dict(
        output=np.asarray(output, dtype=np.float32),
        context=np.asarray(context, dtype=np.float32),
        W_weight=np.asarray(W_weight, dtype=np.float32),
        W_bias=np.asarray(W_bias, dtype=np.float32),
        segment_ids=np.asarray(segment_ids, dtype=np.int32)))
    return out, align


# revision 29
# speedup vs baseline: 1.0150x; 1.0150x over previous
"""Trainium2 Bass kernel for nn_Attn_30820685316537 (segment_reduce attention).

Reference computation (per batch b):
    score = output @ context^T                     [Q, S]
    avg   = per-segment mean of score over S, broadcast back
    align = softmax(avg, axis=S)                   [Q, S]
    ac    = align @ context                        [Q, D]
    out   = tanh(concat(ac, output) @ W^T + bias)  [Q, D]
    returns (out, align)

Everything factors through rank-64 segment space (avg is constant within each
contiguous segment).  With Cavg[n, d] = (1/cnt_n) * sum_{s in seg n} C[s, d]:
    segavg[q, n] = O[q, :] . Cavg[n, :]
    u[q, n]      = exp(segavg + ln cnt_n - max)        (cnt-weighted softmax)
    urn[q, n]    = u / sum_n u                         (per-segment align mass)
    align[q, s]  = urn[q, seg(s)] / cnt_{seg(s)}       (host-side gather)
    ac[q, :]     = urn @ Cavg
    out          = tanh(urn @ (Cavg @ W1) + O @ W2 + bias)
where W^T = [W1; W2].  P = Cavg @ W1 is a [64, D] matrix, so the output GEMM
is O @ W2 (K=1024) plus a K=65 rank-64 correction (bias folded in as P's 65th
row) instead of the K=2048 concat GEMM.

Schedule: all inputs are host-packed into their final SBUF layouts and
streamed need-ordered, each tensor split across the two HW-DGE rings (sync +
scalar) so single-tensor arrival time is halved; identh/bias + w2-hi ride the
software-DGE gpsimd ring.  A memset-backed junk-matmul block warms the PE HAM
clock-gate before any input lands, and small keep-alive matmul groups are
placed at every known input-wait point so the clock never re-throttles during
the input-paced ramp.  Per batch: csum (one-hot matmul, invc pre-folded on
host) -> 8 PE transposes -> segavg -> per-q-tile softmax (DVE/ACT) -> urT
transposes; P = CavgT^T @ W1 in fp8 (x4096 scaling).  qloop per 128-row
q-tile: O@W2 lo then hi columns; urn applied (K=65) into the same PSUM
accumulators; tanh + DMA out per half.  Two batches per core are
software-pipelined; GEMMs fp16 (fp32 PSUM), P-path fp8.  align is
reconstructed on the host by a pure gather of the device-computed urn masses.

Sharding: data-parallel over batch B=16 across 8 NeuronCores; W replicated.
"""
import numpy as np
from contextlib import ExitStack

B, Q, S, D = 16, 512, 1024, 1024
NSEG = 64
NCORES = 8
BPC = B // NCORES          # batches per core
QT = Q // 128              # 4 q-tiles
ST = S // 128              # 8 s-chunks
DT = D // 128              # 8 d-chunks

_CACHE = {}


def _build_nc():
    import concourse.bacc as bacc
    import concourse.tile as tile
    import concourse.mybir as mybir

    f32 = mybir.dt.float32
    f16 = mybir.dt.float16
    f8 = mybir.dt.float8e4

    nc = bacc.Bacc("TRN2", target_bir_lowering=False, debug=False,
                   enable_asserts=False, num_devices=NCORES)

    # All inputs host-packed to final SBUF layout [128, X].
    ctx_in = nc.dram_tensor("ctx_in", [BPC, 128, ST * D], f16, kind="ExternalInput")
    ott_in = nc.dram_tensor("ott_in", [BPC, 128, DT * Q], f16, kind="ExternalInput")
    w1_in = nc.dram_tensor("w1_in", [128, DT * D], f8, kind="ExternalInput")
    w2lo_in = nc.dram_tensor("w2lo_in", [128, DT * 512], f16, kind="ExternalInput")
    w2hi_in = nc.dram_tensor("w2hi_in", [128, DT * 512], f16, kind="ExternalInput")
    ohi_in = nc.dram_tensor("ohi_in", [BPC, 128, ST * NSEG], f16, kind="ExternalInput")
    lnc_in = nc.dram_tensor("lnc_in", [BPC, 128, NSEG], f32, kind="ExternalInput")
    bias_in = nc.dram_tensor("bias_in", [1, D], f16, kind="ExternalInput")
    identh_in = nc.dram_tensor("identh_in", [128, 128], f16, kind="ExternalInput")

    out_o = nc.dram_tensor("out_o", [BPC, Q, D], f16, kind="ExternalOutput")
    urn_o = nc.dram_tensor("urn_o", [BPC, Q, NSEG], f16, kind="ExternalOutput")

    Exp = mybir.ActivationFunctionType.Exp
    Tanh = mybir.ActivationFunctionType.Tanh
    Xax = mybir.AxisListType.X

    with tile.TileContext(nc) as tc, ExitStack() as ectx:
        consts = ectx.enter_context(tc.tile_pool(name="consts", bufs=1))
        inp = ectx.enter_context(tc.tile_pool(name="inp", bufs=2))
        front = ectx.enter_context(tc.tile_pool(name="front", bufs=2))
        sm = ectx.enter_context(tc.tile_pool(name="sm", bufs=3))
        stage = ectx.enter_context(tc.tile_pool(name="stage", bufs=2))

        # PSUM: exactly 8 banks (2 + 1 + 2 + 2 + 1 junk).
        ps64 = ectx.enter_context(tc.tile_pool(name="ps64", bufs=2, space="PSUM"))
        ps_t = ectx.enter_context(tc.tile_pool(name="ps_t", bufs=1, space="PSUM"))
        ps_lo = ectx.enter_context(tc.tile_pool(name="ps_lo", bufs=2, space="PSUM"))
        ps_hi = ectx.enter_context(tc.tile_pool(name="ps_hi", bufs=2, space="PSUM"))
        ps_j = ectx.enter_context(tc.tile_pool(name="ps_j", bufs=1, space="PSUM"))

        # ---- HAM warmup: junk weights via memset (no DMA dependency) ----
        junkw = consts.tile([128, 128], f16, tag="junkw")
        nc.vector.memset(junkw[:], 0.0)
        junkp = ps_j.tile([128, 128], f32, tag="junk", name="junkp")

        def emit_warmup(n, group=True):
            # n back-to-back junk matmuls (N=128) to hold the HAM clock-gate
            # at full rate; `group=True` makes them one accumulation group
            # (single semaphore).
            for r in range(n):
                nc.tensor.matmul(junkp[:], junkw[:], junkw[:],
                                 start=(r == 0 if group else True),
                                 stop=(r == n - 1 if group else True))

        # ---- const/weight tiles ----
        identh = consts.tile([128, 128], f16, tag="identh")
        bias_sb = consts.tile([1, D], f16, tag="bias")
        w1_all = consts.tile([128, DT * D], f8, tag="w1")        # [p, (d f)] fp8
        w2_all = consts.tile([128, 2 * DT * 512], f16, tag="w2")  # lo cols then hi cols
        w2lo = w2_all[:, 0:DT * 512]
        w2hi = w2_all[:, DT * 512:]

        state = [dict() for _ in range(BPC)]

        def alloc_inputs(b):
            st = state[b]
            st["ohi"] = inp.tile([128, ST * NSEG], f16, tag="ohi", name=f"ohi{b}")
            st["lnc"] = inp.tile([128, NSEG], f32, tag="lnc", name=f"lnc{b}")
            st["ctx"] = inp.tile([128, ST * D], f16, tag="ctx", name=f"ctx{b}")
            st["ott"] = inp.tile([128, DT * Q], f16, tag="ott", name=f"ott{b}")

        alloc_inputs(0)
        alloc_inputs(1)

        # ---- DMA schedule: need-ordered, split across both HW rings ----
        # sync ring: first half of every bulk tensor; scalar ring: ohi/lnc
        # first, then second halves.  gpsimd (SW DGE): identh/bias + w2-hi.
        sy, sc, gp = nc.sync, nc.scalar, nc.gpsimd

        gp.dma_start(identh[:], identh_in.ap())
        gp.dma_start(bias_sb[:], bias_in.ap())

        st0, st1 = state[0], state[1]
        sc.dma_start(st0["ohi"][:], ohi_in.ap()[0])
        sc.dma_start(st0["lnc"][:], lnc_in.ap()[0])

        # ctx0: 4 calls of 2 s-chunks each; sync gets chunks 0-3, scalar 4-7.
        for i, eng in ((0, sy), (2, sy), (4, sc), (6, sc)):
            eng.dma_start(st0["ctx"][:, D * i:D * (i + 2)],
                          ctx_in.ap()[0][:, D * i:D * (i + 2)])
        # ott0 halves
        sy.dma_start(st0["ott"][:, 0:4 * Q], ott_in.ap()[0][:, 0:4 * Q])
        sc.dma_start(st0["ott"][:, 4 * Q:], ott_in.ap()[0][:, 4 * Q:])
        # w1 halves
        sy.dma_start(w1_all[:, 0:4 * D], w1_in.ap()[:, 0:4 * D])
        sc.dma_start(w1_all[:, 4 * D:], w1_in.ap()[:, 4 * D:])
        # w2 lo halves (qlo pass), hi on gpsimd (needed ~6us later)
        sy.dma_start(w2lo[:, 0:4 * 512], w2lo_in.ap()[:, 0:4 * 512])
        sc.dma_start(w2lo[:, 4 * 512:], w2lo_in.ap()[:, 4 * 512:])
        gp.dma_start(w2hi[:, 0:4 * 512], w2hi_in.ap()[:, 0:4 * 512])
        gp.dma_start(w2hi[:, 4 * 512:], w2hi_in.ap()[:, 4 * 512:])
        # batch-1 inputs
        sc.dma_start(st1["ohi"][:], ohi_in.ap()[1])
        sc.dma_start(st1["lnc"][:], lnc_in.ap()[1])
        for i, eng in ((0, sy), (2, sy), (4, sc), (6, sc)):
            eng.dma_start(st1["ctx"][:, D * i:D * (i + 2)],
                          ctx_in.ap()[1][:, D * i:D * (i + 2)])
        sy.dma_start(st1["ott"][:, 0:4 * Q], ott_in.ap()[1][:, 0:4 * Q])
        sc.dma_start(st1["ott"][:, 4 * Q:], ott_in.ap()[1][:, 4 * Q:])

        def emit_csum_mm(b, i):
            # col-group pair: chunk i's d-lo -> psum rows 0:64 (col group 0),
            # d-hi -> rows 64:128 (col group 1); one matmul span per chunk.
            st = state[b]
            ohi, ctx_all = st["ohi"], st["ctx"]
            if i == 0:
                st["csp"] = ps64.tile([128, 512], f32, tag="a64", name=f"csp{b}")
            csp = st["csp"]
            oh_i = ohi[:, NSEG * i:NSEG * (i + 1)]
            nc.tensor.matmul(csp[0:64, :], oh_i, ctx_all[:, D * i:D * i + 512],
                             start=(i == 0), stop=(i == ST - 1),
                             tile_position=(0, 0))
            nc.tensor.matmul(csp[64:128, :], oh_i, ctx_all[:, D * i + 512:D * (i + 1)],
                             start=(i == 0), stop=(i == ST - 1),
                             tile_position=(0, 64))

        def emit_csum_tail(b):
            # stacked csum [128, 512] (rows 0:64 = Cavg d-lo, 64:128 = d-hi)
            # -> SBUF; all 8 CavgT chunks come from plain matmuls with
            # identity slices (rhs identh[:, 0:64] reads partitions 0:64,
            # identh[:, 64:128] reads 64:128) — no transpose_mode needed.
            st = state[b]
            css = front.tile([128, 512], f16, tag="css", name=f"css{b}")
            nc.vector.tensor_copy(css[:], st["csp"][:])
            pt = ps_t.tile([128, 512], f32, tag="tp", name=f"pt{b}")
            for d in range(4):
                blk = css[:, 128 * d:128 * (d + 1)]
                nc.tensor.matmul(pt[:, 64 * d:64 * (d + 1)],
                                 blk, identh[:, 0:64], start=True, stop=True)
                nc.tensor.matmul(pt[:, 64 * (d + 4):64 * (d + 5)],
                                 blk, identh[:, 64:128], start=True, stop=True)
            csumt = front.tile([128, DT * NSEG], f16, tag="csumt")
            nc.vector.tensor_copy(csumt[:], pt[:])
            st["csumt"] = csumt
            csumt8 = front.tile([128, DT * NSEG], f8, tag="csumt8")
            nc.vector.tensor_scalar_mul(csumt8[:], pt[:], 16.0)
            st["csumt8"] = csumt8

            urt = front.tile([65, Q], f16, tag="urt")
            nc.vector.memset(urt[64:65, :], 1.0)
            st["urt"] = urt

        def emit_segavg(b):
            # segavgT stacked [128, 256]: rows 0:64 = q 0:256, rows 64:128 =
            # q 256:512 (col-group pair per d-chunk, N=256 each); then the
            # four [q, n] logit blocks via identity-slice matmuls.
            st = state[b]
            csumt, ott, lnc = st["csumt"], st["ott"], st["lnc"]
            sgp = ps64.tile([128, 256], f32, tag="a64", name=f"sgp{b}")
            for d in range(DT):
                ct_d = csumt[:, NSEG * d:NSEG * (d + 1)]
                nc.tensor.matmul(sgp[0:64, :], ct_d,
                                 ott[:, Q * d:Q * d + 256],
                                 start=(d == 0), stop=(d == DT - 1),
                                 tile_position=(0, 0))
                nc.tensor.matmul(sgp[64:128, :], ct_d,
                                 ott[:, Q * d + 256:Q * d + 512],
                                 start=(d == 0), stop=(d == DT - 1),
                                 tile_position=(0, 64))
            sgs = front.tile([128, 256], f16, tag="sgt", name=f"sgs{b}")
            nc.vector.tensor_copy(sgs[:], sgp[:])
            sgtT = ps_t.tile([128, 512], f32, tag="tp", name=f"sgtT{b}")
            for jj in range(2):
                blk = sgs[:, 128 * jj:128 * (jj + 1)]
                nc.tensor.matmul(sgtT[:, 64 * jj:64 * (jj + 1)],
                                 blk, identh[:, 0:64], start=True, stop=True)
                nc.tensor.matmul(sgtT[:, 64 * (jj + 2):64 * (jj + 3)],
                                 blk, identh[:, 64:128], start=True, stop=True)
            for j in range(QT):
                sg2 = sm.tile([128, NSEG], f32, tag="sg2")
                nc.vector.tensor_add(sg2[:], sgtT[:, 64 * j:64 * (j + 1)], lnc[:])
                mx = sm.tile([128, 1], f32, tag="mx")
                nc.vector.reduce_max(mx[:], sg2[:], axis=Xax)
                negmx = sm.tile([128, 1], f32, tag="negmx")
                nc.vector.tensor_scalar_mul(negmx[:], mx[:], -1.0)
                u = sm.tile([128, NSEG], f16, tag="u")
                dsum = sm.tile([128, 1], f32, tag="dsum")
                nc.scalar.activation(u[:], sg2[:], Exp, bias=negmx[:],
                                     accum_out=dsum[:])
                rd = sm.tile([128, 1], f32, tag="rd")
                nc.vector.reciprocal(rd[:], dsum[:])
                urn = sm.tile([128, NSEG], f16, tag="urn", bufs=5)
                nc.vector.tensor_scalar_mul(urn[:], u[:], rd[:])
                nc.scalar.dma_start(urn_o.ap()[b, 128 * j:128 * (j + 1), :], urn[:])
                st[f"urn{j}"] = urn

        def emit_urt(b):
            # urn.T via plain matmul against the full identity (cheaper than
            # transpose_mode): out[f, n] = urn[n, f].
            st = state[b]
            urt = st["urt"]
            pu = ps_t.tile([64, 512], f32, tag="tp", name=f"pu{b}")
            for j in range(QT):
                nc.tensor.matmul(pu[0:64, 128 * j:128 * (j + 1)],
                                 st[f"urn{j}"][:], identh[:],
                                 start=True, stop=True)
            nc.vector.tensor_copy(urt[0:64, :], pu[0:64, 0:512])

        def emit_p(b):
            # stacked P psum: rows 0:64 = P[:, 0:512], rows 64:128 =
            # P[:, 512:1024]; the hi half returns to partition base 0 via an
            # identity-shift matmul (lhsT = identh[:, 64:128]).
            st = state[b]
            c8 = st["csumt8"]
            pp = ps64.tile([128, 512], f32, tag="a64", name=f"pp{b}")
            for d in range(DT):
                ct_d = c8[:, NSEG * d:NSEG * (d + 1)]
                nc.tensor.matmul(pp[0:64, :], ct_d, w1_all[:, D * d:D * d + 512],
                                 start=(d == 0), stop=(d == DT - 1),
                                 tile_position=(0, 0))
                nc.tensor.matmul(pp[64:128, :], ct_d,
                                 w1_all[:, D * d + 512:D * (d + 1)],
                                 start=(d == 0), stop=(d == DT - 1),
                                 tile_position=(0, 64))
            ppsb = front.tile([128, 512], f16, tag="ppsb", name=f"ppsb{b}")
            nc.vector.tensor_scalar_mul(ppsb[:], pp[:], 1.0 / 4096.0)
            psh = ps_t.tile([64, 512], f32, tag="tp", name=f"psh{b}")
            nc.tensor.matmul(psh[:], identh[:, 64:128], ppsb[:],
                             start=True, stop=True)
            paug = front.tile([65, D], f16, tag="paug")
            nc.vector.tensor_copy(paug[0:64, 0:512], ppsb[0:64, :])
            nc.vector.tensor_copy(paug[0:64, 512:1024], psh[:])
            nc.vector.tensor_copy(paug[64:65, :], bias_sb[:])
            st["paug"] = paug

        def emit_qlo(b, j):
            st = state[b]
            ott = st["ott"]
            o_lo = ps_lo.tile([128, 512], f32, tag="po_lo")
            for d in range(DT):
                otd = ott[:, Q * d + 128 * j:Q * d + 128 * (j + 1)]
                nc.tensor.matmul(o_lo[:], otd, w2lo[:, 512 * d:512 * (d + 1)],
                                 start=(d == 0), stop=False)
            st[f"q{j}"] = o_lo

        def emit_qhi(b, j):
            st = state[b]
            ott = st["ott"]
            o_hi = ps_hi.tile([128, 512], f32, tag="po_hi")
            for d in range(DT):
                otd = ott[:, Q * d + 128 * j:Q * d + 128 * (j + 1)]
                nc.tensor.matmul(o_hi[:], otd, w2hi[:, 512 * d:512 * (d + 1)],
                                 start=(d == 0), stop=False)
            st[f"qh{j}"] = o_hi

        def emit_aplo(b, j):
            st = state[b]
            o_lo = st[f"q{j}"]
            urt, paug = st["urt"], st["paug"]
            ua = urt[:, 128 * j:128 * (j + 1)]
            nc.tensor.matmul(o_lo[:], ua, paug[:, 0:512], start=False, stop=True)
            ost = stage.tile([128, 512], f16, tag="ostl")
            nc.scalar.activation(ost[:], o_lo[:], Tanh)
            nc.scalar.dma_start(out_o.ap()[b, 128 * j:128 * (j + 1), 0:512], ost[:])

        def emit_aphi(b, j):
            st = state[b]
            o_hi = st[f"qh{j}"]
            urt, paug = st["urt"], st["paug"]
            ua = urt[:, 128 * j:128 * (j + 1)]
            nc.tensor.matmul(o_hi[:], ua, paug[:, 512:1024], start=False, stop=True)
            ost = stage.tile([128, 512], f16, tag="osth")
            nc.scalar.activation(ost[:], o_hi[:], Tanh)
            nc.scalar.dma_start(out_o.ap()[b, 128 * j:128 * (j + 1), 512:1024], ost[:])

        # ---- emission ----
        # Initial warmup: runs from ~3.5us (preamble end) through the DMA
        # startup latency window; HAM un-throttles ~3.4us into the block.
        emit_warmup(40)
        # csum0 chunk MMs; keep-alive junk between chunk-pair waits.
        for i in range(ST):
            emit_csum_mm(0, i)
            if i % 2 == 1 and i < ST - 1:
                emit_warmup(4, group=False)
        emit_csum_tail(0)
        emit_warmup(6, group=False)   # ott0 wait
        emit_segavg(0)
        emit_warmup(6, group=False)   # w1 wait
        emit_p(0)
        emit_urt(0)
        emit_warmup(6, group=False)   # w2lo wait
        emit_qlo(0, 0)
        emit_aplo(0, 0)
        emit_qlo(0, 1)
        emit_aplo(0, 1)
        emit_qlo(0, 2)
        emit_aplo(0, 2)
        emit_qlo(0, 3)
        emit_aplo(0, 3)
        emit_qhi(0, 0)
        emit_csum_mm(1, 0)
        emit_aphi(0, 0)
        emit_qhi(0, 1)
        emit_csum_mm(1, 1)
        emit_csum_mm(1, 2)
        emit_aphi(0, 1)
        emit_qhi(0, 2)
        emit_csum_mm(1, 3)
        emit_csum_mm(1, 4)
        emit_aphi(0, 2)
        emit_qhi(0, 3)
        emit_csum_mm(1, 5)
        emit_csum_mm(1, 6)
        emit_csum_mm(1, 7)
        emit_aphi(0, 3)
        emit_csum_tail(1)
        emit_segavg(1)
        emit_p(1)
        emit_qlo(1, 0)
        emit_qlo(1, 1)
        emit_urt(1)
        emit_aplo(1, 0)
        emit_qlo(1, 2)
        emit_aplo(1, 1)
        emit_qlo(1, 3)
        emit_aplo(1, 2)
        emit_aplo(1, 3)
        emit_qhi(1, 0)
        emit_aphi(1, 0)
        emit_qhi(1, 1)
        emit_aphi(1, 1)
        emit_qhi(1, 2)
        emit_aphi(1, 2)
        emit_qhi(1, 3)
        emit_aphi(1, 3)

    nc.compile()
    return nc


def _host_prep(output, context, W_weight, W_bias, segment_ids):
    """Shard over batch; fp16 conversion + index/layout prep (no reductions)."""
    import concourse.mybir as mybir
    np_f8 = mybir.dt.np(mybir.dt.float8e4)
    wt = W_weight.T.astype(np.float16)                       # [2D, D]
    w1 = (wt[:D].astype(np.float32) * 256.0).astype(np_f8)   # [D, D] fp8
    w2 = wt[D:]                                              # [D, D] f16

    def packK(a, ncol):
        return np.ascontiguousarray(
            a.reshape(DT, 128, ncol).transpose(1, 0, 2).reshape(128, DT * ncol))
    w1p = packK(w1, D)
    w2lop = packK(w2[:, 0:512], 512)
    w2hip = packK(w2[:, 512:1024], 512)
    biasr = np.ascontiguousarray(W_bias.astype(np.float16)[None, :])
    identh = np.eye(128, dtype=np.float16)

    in_maps, aligns = [], []
    for c in range(NCORES):
        lo = c * BPC
        ohis, lncs, invcs = [], [], []
        for b in range(BPC):
            ids = segment_ids[lo + b].astype(np.int64)       # [S]
            oh = (ids[:, None] == np.arange(NSEG)[None, :]).astype(np.float32)
            cnt = oh.sum(axis=0)                             # [NSEG]
            invc = 1.0 / np.maximum(cnt, 1.0)
            ohi = (oh * invc[None, :]).astype(np.float16)    # [S, NSEG]
            ohis.append(np.ascontiguousarray(
                ohi.reshape(ST, 128, NSEG).transpose(1, 0, 2).reshape(128, ST * NSEG)))
            lnrow = np.where(cnt > 0, np.log(np.maximum(cnt, 1.0)), -1e30)
            lncs.append(np.ascontiguousarray(np.broadcast_to(
                lnrow.astype(np.float32)[None, :], (128, NSEG))))
            invcs.append(invc)
        ctxp = np.stack([np.ascontiguousarray(
            context[lo + b].astype(np.float16).reshape(ST, 128, D)
            .transpose(1, 0, 2).reshape(128, ST * D)) for b in range(BPC)])
        ottp = np.stack([np.ascontiguousarray(
            output[lo + b].astype(np.float16).T.reshape(DT, 128, Q)
            .transpose(1, 0, 2).reshape(128, DT * Q)) for b in range(BPC)])
        in_maps.append({
            "ctx_in": ctxp, "ott_in": ottp,
            "w1_in": w1p, "w2lo_in": w2lop, "w2hi_in": w2hip,
            "bias_in": biasr, "identh_in": identh,
            "ohi_in": np.stack(ohis), "lnc_in": np.stack(lncs),
        })
        aligns.append(invcs)
    return in_maps, aligns


def _run(inputs, trace=False, tmpdir=None):
    from concourse.bass_utils import run_bass_kernel_spmd
    if "nc" not in _CACHE:
        _CACHE["nc"] = _build_nc()
    nc = _CACHE["nc"]
    in_maps, invcs = _host_prep(**inputs)
    kw = {}
    if trace:
        kw = {"trace": True, "tmpdir": tmpdir}
    res = run_bass_kernel_spmd(nc, in_maps, core_ids=list(range(NCORES)), **kw)
    out = np.concatenate(
        [res.results[c]["out_o"].astype(np.float32) for c in range(NCORES)], axis=0)
    # align[q, s] = urn[q, seg(s)] * invc[seg(s)]  — host-side gather/unshard
    seg = inputs["segment_ids"]
    align = np.empty((B, Q, S), dtype=np.float32)
    for c in range(NCORES):
        for b in range(BPC):
            gb = c * BPC + b
            urn = res.results[c]["urn_o"][b].astype(np.float32)   # [Q, NSEG]
            scaled = urn * invcs[c][b][None, :].astype(np.float32)
            align[gb] = scaled[:, seg[gb].astype(np.int64)]
    return (out, align), res


def kernel(output, context, W_weight, W_bias, segment_ids):
    # Force host numpy up front: if the caller hands us jax arrays, numpy
    # ops would otherwise dispatch to the accelerator backend.
    (out, align), _ = _run(dict(
        output=np.asarray(output, dtype=np.float32),
        context=np.asarray(context, dtype=np.float32),
        W_weight=np.asarray(W_weight, dtype=np.float32),
        W_bias=np.asarray(W_bias, dtype=np.float32),
        segment_ids=np.asarray(segment_ids, dtype=np.int32)))
    return out, align
